# revision 16
# baseline (speedup 1.0000x reference)
"""CoarseMatching (dual-softmax matching) Trainium2 kernel.

Distribution: 8 NeuronCores = 2 batches x 4 L-shards of 1200 rows.
Core c handles batch n = c // 4, rows [ (c%4)*1200, (c%4+1)*1200 ).

Per core:
  phase 1: sim = f0s @ f1^T (bf16 matmuls, K=256 in 2 chunks, S in 10 chunks
           of 480 = one PSUM bank), E = exp(sim) via ACT with accum_out giving
           row-sum partials; column sums via ones^T @ E matmul accumulated in
           PSUM; two AllReduce(add) over the 4 L-shard cores (one per S-half,
           pipelined behind compute).
  phase 2: recompute sim per tile with a 3rd K=2 augmented matmul chunk that
           adds -0.5*log(colsum_global) (split hi+lo bf16 for f32-level
           accuracy); conf = Exp(2*x + bias=-log(rowsum)) straight from ACT;
           DVE accumulates column-max (mutual-NN check) and row max/argmax;
           conf chunks DMA to DRAM as they complete.
  tail:    gpsimd partition_all_reduce + AllReduce(max) for global colmax.

Host: shard/transpose/cast inputs, then threshold/border/mutual-NN logic on
the tiny [N, L] vectors (device supplies rowmax, row argmax, global colmax).
"""
import os
import sys

for _p in ("/opt/trn_rl_repo", "/root/.axon_site/_ro/trn_rl_repo"):
    if os.path.isdir(_p) and _p not in sys.path:
        sys.path.insert(0, _p)

import numpy as np
import ml_dtypes

import concourse.bass as bass
import concourse.bacc as bacc
import concourse.tile as tile
import concourse.mybir as mybir
import concourse.bass_isa as bass_isa
from concourse import bass_utils

# Problem constants (hardcoded per harness contract)
N, L, S, C = 2, 4800, 4800, 256
THR, TEMPERATURE, BORDER_RM = 0.2, 0.1, 2
H0C, W0C, H1C, W1C = 60, 80, 60, 80
NCORES = 8
NSH = 4              # L-shards per batch
LSH = L // NSH       # 1200 rows per core
PT = 120             # partition-tile rows
NT = LSH // PT       # 10 l-tiles per core
SCH = 480            # S chunk (one PSUM bank of f32)
NCH = S // SCH       # 10 chunks
KCH = C // 128       # 2 contraction chunks
HALF = NCH // 2      # chunks per colsum collective batch

F32 = mybir.dt.float32
BF16 = mybir.dt.bfloat16
U32 = mybir.dt.uint32
AF = mybir.ActivationFunctionType
ALU = mybir.AluOpType

_CACHE = {}


def _build_module():
    nc = bacc.Bacc("TRN2", target_bir_lowering=False, debug=False,
                   enable_asserts=True, num_devices=NCORES)

    f0t_in = nc.dram_tensor("f0t", [C, LSH], BF16, kind="ExternalInput")
    f1t_in = nc.dram_tensor("f1t", [C, S], BF16, kind="ExternalInput")
    conf_out = nc.dram_tensor("conf", [LSH, S], F32, kind="ExternalOutput")
    rowmax_out = nc.dram_tensor("rowmax", [LSH], F32, kind="ExternalOutput")
    jstar_out = nc.dram_tensor("jstar", [LSH], U32, kind="ExternalOutput")
    colmax_out = nc.dram_tensor("colmax", [S], F32, kind="ExternalOutput")

    # collective bounce buffers (internal DRAM); one colsum collective per S-half
    cs_in = [nc.dram_tensor(f"cs_in{h}", [1, S // 2], F32, kind="Internal")
             for h in range(2)]
    cs_out = [nc.dram_tensor(f"cs_out{h}", [1, S // 2], F32, kind="Internal")
              for h in range(2)]
    cm_in = nc.dram_tensor("cm_in", [1, S], F32, kind="Internal")
    cm_out = nc.dram_tensor("cm_out", [1, S], F32, kind="Internal")
    rg = [[0, 1, 2, 3], [4, 5, 6, 7]]

    with tile.TileContext(nc) as tc:
        with (
            tc.tile_pool(name="const", bufs=1) as cpool,
            tc.tile_pool(name="stats", bufs=1) as spool,
            tc.tile_pool(name="ework", bufs=4) as epool,
            tc.tile_pool(name="confp", bufs=3) as confpool,
            tc.tile_pool(name="cmaxp", bufs=1) as cmpool,
            tc.tile_pool(name="vec", bufs=2) as vpool,
            tc.tile_pool(name="psim", bufs=3, space="PSUM") as psim_pool,
            tc.tile_pool(name="pcol", bufs=2, space="PSUM") as pcol_pool,
        ):
            # ---- load operands ----
            f0t = cpool.tile([128, KCH * LSH], BF16)    # K-chunks side by side
            f1t = cpool.tile([128, KCH * S], BF16)
            for k in range(KCH):
                nc.sync.dma_start(f0t[:, k * LSH:(k + 1) * LSH],
                                  f0t_in[k * 128:(k + 1) * 128, :])
                nc.sync.dma_start(f1t[:, k * S:(k + 1) * S],
                                  f1t_in[k * 128:(k + 1) * 128, :])
            ones_bf = cpool.tile([PT, 1], BF16)
            nc.vector.memset(ones_bf[:], 1.0)
            ones2 = cpool.tile([2, PT], BF16)           # lhsT for aug chunk
            nc.vector.memset(ones2[:], 1.0)

            # f1 augmented rows: hi/lo bf16 of -0.5*log(colsum_global)
            f1aug = cpool.tile([2, S], BF16)

            # ---- stats tiles ----
            rs_part = spool.tile([PT, NT * NCH], F32)   # rowsum partials
            neglog_rs = spool.tile([PT, NT], F32)       # -log rowsum per tile
            rowsum = spool.tile([PT, NT], F32)
            jst = spool.tile([PT, 8], U32)
            rowmax_all = spool.tile([PT, NT], F32)

            colmax_run = cmpool.tile([PT, S], F32)
            nc.vector.memset(colmax_run[:], 0.0)

            # =================== PHASE 1: stats ===================
            for c in range(NCH):
                pcol = pcol_pool.tile([1, SCH], F32, tag="pcol")
                for i in range(NT):
                    ps = psim_pool.tile([PT, SCH], F32, tag="psim")
                    for k in range(KCH):
                        nc.tensor.matmul(
                            ps[:, :],
                            f0t[:, k * LSH + i * PT: k * LSH + (i + 1) * PT],
                            f1t[:, k * S + c * SCH: k * S + (c + 1) * SCH],
                            start=(k == 0), stop=(k == KCH - 1),
                        )
                    e_bf = epool.tile([PT, SCH], BF16, tag="E")
                    nc.scalar.activation(
                        e_bf[:], ps[:], AF.Exp,
                        accum_out=rs_part[:, i * NCH + c: i * NCH + c + 1])
                    nc.tensor.matmul(pcol[:, :], ones_bf[:], e_bf[:, :],
                                     start=(i == 0), stop=(i == NT - 1))
                # local chunk colsum -> staging DRAM for the half-collective
                cs_sb = vpool.tile([1, SCH], F32, tag="cs_sb")
                nc.scalar.copy(cs_sb[:], pcol[:])
                h = c // HALF
                off = (c % HALF) * SCH
                nc.sync.dma_start(cs_in[h][0:1, off:off + SCH], cs_sb[:])

                if c % HALF == HALF - 1:
                    # one AllReduce per S-half, then batched Ln + hi/lo split
                    nc.gpsimd.collective_compute(
                        "AllReduce", ALU.add, replica_groups=rg,
                        ins=[cs_in[h].ap()], outs=[cs_out[h].ap()])
                    W = S // 2
                    v = vpool.tile([1, W], F32, tag="v")
                    m = vpool.tile([1, W], F32, tag="m")
                    hi_bf = vpool.tile([1, W], BF16, tag="hib")
                    lo_bf = vpool.tile([1, W], BF16, tag="lob")
                    nc.sync.dma_start(v[:, :], cs_out[h][:, :])
                    nc.scalar.activation(m[:], v[:], AF.Ln)
                    nc.vector.tensor_scalar_mul(m[:], m[:], -0.5)
                    nc.vector.tensor_copy(hi_bf[:], m[:])
                    nc.vector.tensor_copy(v[:], hi_bf[:])
                    nc.vector.tensor_tensor(m[:], m[:], v[:], ALU.subtract)
                    nc.vector.tensor_copy(lo_bf[:], m[:])
                    sl = slice(h * W, (h + 1) * W)
                    nc.sync.dma_start(f1aug[0:1, sl], hi_bf[:])
                    nc.sync.dma_start(f1aug[1:2, sl], lo_bf[:])

            # rowsums + -log(rowsum) per tile
            for i in range(NT):
                nc.vector.reduce_sum(
                    rowsum[:, i:i + 1],
                    rs_part[:, i * NCH:(i + 1) * NCH],
                    axis=mybir.AxisListType.X)
            nc.scalar.activation(neglog_rs[:], rowsum[:], AF.Ln)
            nc.vector.tensor_scalar_mul(neglog_rs[:], neglog_rs[:], -1.0)

            # =================== PHASE 2: conf + outputs ===================
            for i in range(NT):
                conf_i = confpool.tile([PT, S], F32, tag="conf")
                for c in range(NCH):
                    ps2 = psim_pool.tile([PT, SCH], F32, tag="psim")
                    for k in range(KCH):
                        nc.tensor.matmul(
                            ps2[:, :],
                            f0t[:, k * LSH + i * PT: k * LSH + (i + 1) * PT],
                            f1t[:, k * S + c * SCH: k * S + (c + 1) * SCH],
                            start=(k == 0), stop=False,
                        )
                    nc.tensor.matmul(
                        ps2[:, :], ones2[:, :],
                        f1aug[:, c * SCH:(c + 1) * SCH],
                        start=False, stop=True,
                    )
                    sl = slice(c * SCH, (c + 1) * SCH)
                    # conf = exp(2*(sim - 0.5 log colsum) - log rowsum)
                    nc.scalar.activation(conf_i[:, sl], ps2[:], AF.Exp,
                                         bias=neglog_rs[:, i:i + 1],
                                         scale=2.0)
                    if i == NT - 1:
                        # final tile: finish each colmax chunk and start its
                        # cross-partition reduce right away so the tail
                        # overlaps the remaining chunks' compute
                        nc.vector.tensor_tensor(colmax_run[:, sl],
                                                colmax_run[:, sl],
                                                conf_i[:, sl], ALU.max)
                        par_sm = vpool.tile([PT, SCH], F32, tag="par")
                        nc.gpsimd.partition_all_reduce(
                            par_sm[:], colmax_run[:, sl], channels=PT,
                            reduce_op=bass_isa.ReduceOp.max)
                        nc.sync.dma_start(cm_in[0:1, sl], par_sm[0:1, :])
                # one large DMA per tile: 19.2KB contiguous bursts per row
                nc.sync.dma_start(conf_out[i * PT:(i + 1) * PT, :], conf_i[:])
                if i < NT - 1:
                    # column-max accumulation, one full-width op per tile
                    nc.vector.tensor_tensor(colmax_run[:], colmax_run[:],
                                            conf_i[:], ALU.max)
                # row max + argmax
                nc.vector.reduce_max(rowmax_all[:, i:i + 1], conf_i[:],
                                     axis=mybir.AxisListType.X)
                nc.vector.max_index(
                    jst[:], rowmax_all[:, i:i + 1].broadcast_to((PT, 8)),
                    conf_i[:])
                nc.sync.dma_start(rowmax_out[i * PT:(i + 1) * PT],
                                  rowmax_all[:, i:i + 1])
                nc.sync.dma_start(jstar_out[i * PT:(i + 1) * PT],
                                  jst[:, 0:1])

            # =================== TAIL: global colmax ===================
            nc.gpsimd.collective_compute(
                "AllReduce", ALU.max, replica_groups=rg,
                ins=[cm_in.ap()], outs=[cm_out.ap()])
            nc.sync.dma_start(colmax_out[:], cm_out[0:1, :])

    nc.compile()
    return nc


def _border_mask_1d(h, w, b):
    ih = np.arange(h)
    iw = np.arange(w)
    mh = (ih >= b) & (ih < h - b)
    mw = (iw >= b) & (iw < w - b)
    return (mh[:, None] & mw[None, :]).reshape(-1)


def kernel(feat_c0, feat_c1):
    feat_c0 = np.asarray(feat_c0, dtype=np.float32)
    feat_c1 = np.asarray(feat_c1, dtype=np.float32)

    if "nc" not in _CACHE:
        _CACHE["nc"] = _build_module()
    nc = _CACHE["nc"]

    kscale = np.float32(1.0 / (C * TEMPERATURE))  # folded into f0
    in_maps = []
    for core in range(NCORES):
        n, sh = core // NSH, core % NSH
        f0_sl = feat_c0[n, sh * LSH:(sh + 1) * LSH, :]  # [LSH, C]
        f0t = np.ascontiguousarray((f0_sl * kscale).T).astype(ml_dtypes.bfloat16)
        f1t = np.ascontiguousarray(feat_c1[n].T).astype(ml_dtypes.bfloat16)
        in_maps.append({"f0t": f0t, "f1t": f1t})

    res = bass_utils.run_bass_kernel_spmd(nc, in_maps, core_ids=list(range(NCORES)))
    results = res.results

    conf = np.empty((N, L, S), np.float32)
    rowmax = np.empty((N, L), np.float32)
    jstar = np.empty((N, L), np.int64)
    colmax = np.empty((N, S), np.float32)
    for core in range(NCORES):
        n, sh = core // NSH, core % NSH
        r = results[core]
        conf[n, sh * LSH:(sh + 1) * LSH, :] = r["conf"]
        rowmax[n, sh * LSH:(sh + 1) * LSH] = r["rowmax"]
        jstar[n, sh * LSH:(sh + 1) * LSH] = r["jstar"].astype(np.int64)
        if sh == 0:
            colmax[n] = r["colmax"]

    # host finalize: threshold/border/mutual-NN on [N, L] vectors
    bm0 = _border_mask_1d(H0C, W0C, BORDER_RM)
    bm1 = _border_mask_1d(H1C, W1C, BORDER_RM)
    jcl = np.clip(jstar, 0, S - 1)
    mask_v = (
        (rowmax > THR)
        & bm0[None, :]
        & bm1[jcl]
        & (rowmax == np.take_along_axis(colmax, jcl, axis=1))
    )
    j_ids = np.where(mask_v, jcl, 0).astype(np.int32)
    mconf = np.where(mask_v, rowmax, 0.0).astype(np.float32)
    return conf, mask_v, j_ids, mconf


if __name__ == "__main__":
    rng = np.random.default_rng(0)
    outs = kernel(feat_c0=rng.standard_normal((N, L, C)).astype(np.float32),
                  feat_c1=rng.standard_normal((N, S, C)).astype(np.float32))
    print([np.asarray(o).shape for o in outs])


# revision 17
# speedup vs baseline: 1.0766x; 1.0766x over previous
"""CoarseMatching (dual-softmax matching) Trainium2 kernel.

Distribution: 8 NeuronCores = 2 batches x 4 L-shards of 1200 rows.
Core c handles batch n = c // 4, rows [ (c%4)*1200, (c%4+1)*1200 ).

Per core:
  phase 1: sim = f0s @ f1^T (bf16 matmuls, K=256 in 2 chunks, S in 10 chunks
           of 480 = one PSUM bank), E = exp(sim) via ACT with accum_out giving
           row-sum partials; column sums via ones^T @ E matmul accumulated in
           PSUM; two AllReduce(add) over the 4 L-shard cores (one per S-half,
           pipelined behind compute).
  phase 2: recompute sim per tile with a 3rd K=2 augmented matmul chunk that
           adds -0.5*log(colsum_global) (split hi+lo bf16 for f32-level
           accuracy); conf = Exp(2*x + bias=-log(rowsum)) straight from ACT;
           DVE accumulates column-max (mutual-NN check) and row max/argmax;
           conf chunks DMA to DRAM as they complete.
  tail:    gpsimd partition_all_reduce + AllReduce(max) for global colmax.

Host: shard/transpose/cast inputs, then threshold/border/mutual-NN logic on
the tiny [N, L] vectors (device supplies rowmax, row argmax, global colmax).
"""
import os
import sys

for _p in ("/opt/trn_rl_repo", "/root/.axon_site/_ro/trn_rl_repo"):
    if os.path.isdir(_p) and _p not in sys.path:
        sys.path.insert(0, _p)

import numpy as np
import ml_dtypes

import concourse.bass as bass
import concourse.bacc as bacc
import concourse.tile as tile
import concourse.mybir as mybir
import concourse.bass_isa as bass_isa
from concourse import bass_utils

# Problem constants (hardcoded per harness contract)
N, L, S, C = 2, 4800, 4800, 256
THR, TEMPERATURE, BORDER_RM = 0.2, 0.1, 2
H0C, W0C, H1C, W1C = 60, 80, 60, 80
NCORES = 8
NSH = 4              # L-shards per batch
LSH = L // NSH       # 1200 rows per core
PT = 120             # partition-tile rows
NT = LSH // PT       # 10 l-tiles per core
SCH = 480            # S chunk (one PSUM bank of f32)
NCH = S // SCH       # 10 chunks
KCH = C // 128       # 2 contraction chunks
HALF = NCH // 2      # chunks per colsum collective batch

F32 = mybir.dt.float32
BF16 = mybir.dt.bfloat16
U32 = mybir.dt.uint32
AF = mybir.ActivationFunctionType
ALU = mybir.AluOpType

_CACHE = {}


def _build_module():
    nc = bacc.Bacc("TRN2", target_bir_lowering=False, debug=False,
                   enable_asserts=True, num_devices=NCORES)

    f0t_in = nc.dram_tensor("f0t", [C, LSH], BF16, kind="ExternalInput")
    f1t_in = nc.dram_tensor("f1t", [C, S], BF16, kind="ExternalInput")
    conf_out = nc.dram_tensor("conf", [LSH, S], F32, kind="ExternalOutput")
    rowmax_out = nc.dram_tensor("rowmax", [LSH], F32, kind="ExternalOutput")
    jstar_out = nc.dram_tensor("jstar", [LSH], U32, kind="ExternalOutput")
    colmax_out = nc.dram_tensor("colmax", [S], F32, kind="ExternalOutput")

    # collective bounce buffers (internal DRAM); one colsum collective per S-half
    cs_in = [nc.dram_tensor(f"cs_in{h}", [1, S // 2], F32, kind="Internal")
             for h in range(2)]
    cs_out = [nc.dram_tensor(f"cs_out{h}", [1, S // 2], F32, kind="Internal")
              for h in range(2)]
    cm_in = nc.dram_tensor("cm_in", [1, S], F32, kind="Internal")
    cm_out = nc.dram_tensor("cm_out", [1, S], F32, kind="Internal")
    rg = [[0, 1, 2, 3], [4, 5, 6, 7]]

    with tile.TileContext(nc) as tc:
        with (
            tc.tile_pool(name="const", bufs=1) as cpool,
            tc.tile_pool(name="stats", bufs=1) as spool,
            tc.tile_pool(name="ework", bufs=4) as epool,
            tc.tile_pool(name="confp", bufs=3) as confpool,
            tc.tile_pool(name="cmaxp", bufs=1) as cmpool,
            tc.tile_pool(name="vec", bufs=2) as vpool,
            tc.tile_pool(name="psim", bufs=3, space="PSUM") as psim_pool,
            tc.tile_pool(name="pcol", bufs=2, space="PSUM") as pcol_pool,
        ):
            # ---- load operands ----
            f0t = cpool.tile([128, KCH * LSH], BF16)    # K-chunks side by side
            f1t = cpool.tile([128, KCH * S], BF16)
            for k in range(KCH):
                nc.sync.dma_start(f0t[:, k * LSH:(k + 1) * LSH],
                                  f0t_in[k * 128:(k + 1) * 128, :])
                nc.sync.dma_start(f1t[:, k * S:(k + 1) * S],
                                  f1t_in[k * 128:(k + 1) * 128, :])
            ones_bf = cpool.tile([PT, 1], BF16)
            nc.vector.memset(ones_bf[:], 1.0)
            ones2 = cpool.tile([2, PT], BF16)           # lhsT for aug chunk
            nc.vector.memset(ones2[:], 1.0)

            # f1 augmented rows: hi/lo bf16 of -0.5*log(colsum_global)
            f1aug = cpool.tile([2, S], BF16)

            # ---- stats tiles ----
            rs_part = spool.tile([PT, NT * NCH], F32)   # rowsum partials
            neglog_rs = spool.tile([PT, NT], F32)       # -log rowsum per tile
            rowsum = spool.tile([PT, NT], F32)
            jst = spool.tile([PT, 8], U32)
            rowmax_all = spool.tile([PT, NT], F32)

            colmax_run = cmpool.tile([PT, S], F32)
            nc.vector.memset(colmax_run[:], 0.0)

            # =================== PHASE 1: stats ===================
            for c in range(NCH):
                pcol = pcol_pool.tile([1, SCH], F32, tag="pcol")
                for i in range(NT):
                    ps = psim_pool.tile([PT, SCH], F32, tag="psim")
                    for k in range(KCH):
                        nc.tensor.matmul(
                            ps[:, :],
                            f0t[:, k * LSH + i * PT: k * LSH + (i + 1) * PT],
                            f1t[:, k * S + c * SCH: k * S + (c + 1) * SCH],
                            start=(k == 0), stop=(k == KCH - 1),
                        )
                    e_bf = epool.tile([PT, SCH], BF16, tag="E")
                    nc.scalar.activation(e_bf[:], ps[:], AF.Exp)
                    # row-sum partial on the otherwise-idle DVE (frees ACT)
                    nc.vector.reduce_sum(
                        rs_part[:, i * NCH + c: i * NCH + c + 1], e_bf[:],
                        axis=mybir.AxisListType.X)
                    nc.tensor.matmul(pcol[:, :], ones_bf[:], e_bf[:, :],
                                     start=(i == 0), stop=(i == NT - 1))
                # local chunk colsum -> staging DRAM for the half-collective
                cs_sb = vpool.tile([1, SCH], F32, tag="cs_sb")
                nc.scalar.copy(cs_sb[:], pcol[:])
                h = c // HALF
                off = (c % HALF) * SCH
                nc.sync.dma_start(cs_in[h][0:1, off:off + SCH], cs_sb[:])

                if c % HALF == HALF - 1:
                    # one AllReduce per S-half, then batched Ln + hi/lo split
                    nc.gpsimd.collective_compute(
                        "AllReduce", ALU.add, replica_groups=rg,
                        ins=[cs_in[h].ap()], outs=[cs_out[h].ap()])
                    W = S // 2
                    v = vpool.tile([1, W], F32, tag="v")
                    m = vpool.tile([1, W], F32, tag="m")
                    hi_bf = vpool.tile([1, W], BF16, tag="hib")
                    lo_bf = vpool.tile([1, W], BF16, tag="lob")
                    nc.sync.dma_start(v[:, :], cs_out[h][:, :])
                    nc.scalar.activation(m[:], v[:], AF.Ln)
                    nc.vector.tensor_scalar_mul(m[:], m[:], -0.5)
                    nc.vector.tensor_copy(hi_bf[:], m[:])
                    nc.vector.tensor_copy(v[:], hi_bf[:])
                    nc.vector.tensor_tensor(m[:], m[:], v[:], ALU.subtract)
                    nc.vector.tensor_copy(lo_bf[:], m[:])
                    sl = slice(h * W, (h + 1) * W)
                    nc.sync.dma_start(f1aug[0:1, sl], hi_bf[:])
                    nc.sync.dma_start(f1aug[1:2, sl], lo_bf[:])

            # rowsums + -log(rowsum) per tile
            for i in range(NT):
                nc.vector.reduce_sum(
                    rowsum[:, i:i + 1],
                    rs_part[:, i * NCH:(i + 1) * NCH],
                    axis=mybir.AxisListType.X)
            nc.scalar.activation(neglog_rs[:], rowsum[:], AF.Ln)
            nc.vector.tensor_scalar_mul(neglog_rs[:], neglog_rs[:], -1.0)

            # =================== PHASE 2: conf + outputs ===================
            for i in range(NT):
                conf_i = confpool.tile([PT, S], F32, tag="conf")
                for c in range(NCH):
                    ps2 = psim_pool.tile([PT, SCH], F32, tag="psim")
                    for k in range(KCH):
                        nc.tensor.matmul(
                            ps2[:, :],
                            f0t[:, k * LSH + i * PT: k * LSH + (i + 1) * PT],
                            f1t[:, k * S + c * SCH: k * S + (c + 1) * SCH],
                            start=(k == 0), stop=False,
                        )
                    nc.tensor.matmul(
                        ps2[:, :], ones2[:, :],
                        f1aug[:, c * SCH:(c + 1) * SCH],
                        start=False, stop=True,
                    )
                    sl = slice(c * SCH, (c + 1) * SCH)
                    # conf = exp(2*(sim - 0.5 log colsum) - log rowsum)
                    nc.scalar.activation(conf_i[:, sl], ps2[:], AF.Exp,
                                         bias=neglog_rs[:, i:i + 1],
                                         scale=2.0)
                    if i == NT - 1:
                        # final tile: finish each colmax chunk and start its
                        # cross-partition reduce right away so the tail
                        # overlaps the remaining chunks' compute
                        nc.vector.tensor_tensor(colmax_run[:, sl],
                                                colmax_run[:, sl],
                                                conf_i[:, sl], ALU.max)
                        par_sm = vpool.tile([PT, SCH], F32, tag="par")
                        nc.gpsimd.partition_all_reduce(
                            par_sm[:], colmax_run[:, sl], channels=PT,
                            reduce_op=bass_isa.ReduceOp.max)
                        nc.sync.dma_start(cm_in[0:1, sl], par_sm[0:1, :])
                # one large DMA per tile: 19.2KB contiguous bursts per row
                nc.sync.dma_start(conf_out[i * PT:(i + 1) * PT, :], conf_i[:])
                if i < NT - 1:
                    # column-max accumulation, one full-width op per tile
                    nc.vector.tensor_tensor(colmax_run[:], colmax_run[:],
                                            conf_i[:], ALU.max)
                # row max + argmax
                nc.vector.reduce_max(rowmax_all[:, i:i + 1], conf_i[:],
                                     axis=mybir.AxisListType.X)
                nc.vector.max_index(
                    jst[:], rowmax_all[:, i:i + 1].broadcast_to((PT, 8)),
                    conf_i[:])
                nc.sync.dma_start(rowmax_out[i * PT:(i + 1) * PT],
                                  rowmax_all[:, i:i + 1])
                nc.sync.dma_start(jstar_out[i * PT:(i + 1) * PT],
                                  jst[:, 0:1])

            # =================== TAIL: global colmax ===================
            nc.gpsimd.collective_compute(
                "AllReduce", ALU.max, replica_groups=rg,
                ins=[cm_in.ap()], outs=[cm_out.ap()])
            nc.sync.dma_start(colmax_out[:], cm_out[0:1, :])

    nc.compile()
    return nc


def _border_mask_1d(h, w, b):
    ih = np.arange(h)
    iw = np.arange(w)
    mh = (ih >= b) & (ih < h - b)
    mw = (iw >= b) & (iw < w - b)
    return (mh[:, None] & mw[None, :]).reshape(-1)


def kernel(feat_c0, feat_c1):
    feat_c0 = np.asarray(feat_c0, dtype=np.float32)
    feat_c1 = np.asarray(feat_c1, dtype=np.float32)

    if "nc" not in _CACHE:
        _CACHE["nc"] = _build_module()
    nc = _CACHE["nc"]

    kscale = np.float32(1.0 / (C * TEMPERATURE))  # folded into f0
    in_maps = []
    for core in range(NCORES):
        n, sh = core // NSH, core % NSH
        f0_sl = feat_c0[n, sh * LSH:(sh + 1) * LSH, :]  # [LSH, C]
        f0t = np.ascontiguousarray((f0_sl * kscale).T).astype(ml_dtypes.bfloat16)
        f1t = np.ascontiguousarray(feat_c1[n].T).astype(ml_dtypes.bfloat16)
        in_maps.append({"f0t": f0t, "f1t": f1t})

    res = bass_utils.run_bass_kernel_spmd(nc, in_maps, core_ids=list(range(NCORES)))
    results = res.results

    conf = np.empty((N, L, S), np.float32)
    rowmax = np.empty((N, L), np.float32)
    jstar = np.empty((N, L), np.int64)
    colmax = np.empty((N, S), np.float32)
    for core in range(NCORES):
        n, sh = core // NSH, core % NSH
        r = results[core]
        conf[n, sh * LSH:(sh + 1) * LSH, :] = r["conf"]
        rowmax[n, sh * LSH:(sh + 1) * LSH] = r["rowmax"]
        jstar[n, sh * LSH:(sh + 1) * LSH] = r["jstar"].astype(np.int64)
        if sh == 0:
            colmax[n] = r["colmax"]

    # host finalize: threshold/border/mutual-NN on [N, L] vectors
    bm0 = _border_mask_1d(H0C, W0C, BORDER_RM)
    bm1 = _border_mask_1d(H1C, W1C, BORDER_RM)
    jcl = np.clip(jstar, 0, S - 1)
    mask_v = (
        (rowmax > THR)
        & bm0[None, :]
        & bm1[jcl]
        & (rowmax == np.take_along_axis(colmax, jcl, axis=1))
    )
    j_ids = np.where(mask_v, jcl, 0).astype(np.int32)
    mconf = np.where(mask_v, rowmax, 0.0).astype(np.float32)
    return conf, mask_v, j_ids, mconf


if __name__ == "__main__":
    rng = np.random.default_rng(0)
    outs = kernel(feat_c0=rng.standard_normal((N, L, C)).astype(np.float32),
                  feat_c1=rng.standard_normal((N, S, C)).astype(np.float32))
    print([np.asarray(o).shape for o in outs])
